# revision 1
# baseline (speedup 1.0000x reference)
"""Trainium2 Bass kernel for nn_CNNMnist_Sketch (sketched CNN forward pass).

Data-parallel over 8 NeuronCores: batch 4096 -> 512 per core.
Per-core pipeline (all shapes hardcoded):
  conv1 5x5 (1->32ch) + maxpool2 + relu   -> h1  [32ch, 12x12]
  conv2 5x5 (32->64ch) + maxpool2 + relu  -> h2  [64ch, 4x4] -> flat 1024
  fc1 1024->512 + relu, fc2 512->10, log_softmax

Key layout tricks:
  - conv1: input replicated to 100 SBUF partitions (4 batch-chunks x 25 taps),
    each partition pre-shifted by its tap offset; a single block-diagonal
    [100,128] lhsT computes 4 chunks x 32 channels in one matmul stream.
  - conv2: pooled h1 replicated to 128 partitions (4 kw-shifted copies x 32ch)
    so 4 taps contract per pass (5 K=128 passes + 5 K=32 passes for kw=4).
  - fc2/log_softmax run with batch on partitions -> free-dim reductions.
"""

import numpy as np
import ml_dtypes

import concourse.bass as bass
import concourse.bacc as bacc
import concourse.tile as tile
from concourse import mybir
from concourse.bass_utils import run_bass_kernel_spmd

F32 = mybir.dt.float32
F32R = mybir.dt.float32r
BF16 = mybir.dt.bfloat16
RELU = mybir.ActivationFunctionType.Relu
EXP = mybir.ActivationFunctionType.Exp
LN = mybir.ActivationFunctionType.Ln
MAXOP = mybir.AluOpType.max
SUBOP = mybir.AluOpType.subtract
ADDOP = mybir.AluOpType.add
AXY = mybir.AxisListType.XY
AX = mybir.AxisListType.X

NCORES = 8
BPC = 4096 // NCORES          # samples per core
BLK = 64                      # samples per block
NBLK = BPC // BLK
CS = BLK // 4                 # samples per conv1 chunk (4 chunks / block)
CHUNKF = CS * 784             # x elements per chunk
XBLK = BLK * 784              # x elements per block
H1F = CS * 144                # h1 elements per chunk (per channel)
XPAD = 128                    # DRAM pad so shifted reads never go OOB

_CACHE = {}


def _build():
    nc = bacc.Bacc(target_bir_lowering=False, debug=False, num_devices=NCORES)

    xt = nc.dram_tensor("x", [BPC * 784 + XPAD], BF16, kind="ExternalInput").ap()
    wc1t = nc.dram_tensor("wc1bd", [100, 128], BF16, kind="ExternalInput").ap()
    w2at = nc.dram_tensor("w2a", [128, 5 * 64], BF16, kind="ExternalInput").ap()
    w2bt = nc.dram_tensor("w2b", [32, 5 * 64], BF16, kind="ExternalInput").ap()
    w3t = nc.dram_tensor("w3sb", [128, 4096], BF16, kind="ExternalInput").ap()
    fc2t = nc.dram_tensor("fc2sb", [128, 40], F32, kind="ExternalInput").ap()
    b1t = nc.dram_tensor("b1r", [128, 1], F32, kind="ExternalInput").ap()
    b2t = nc.dram_tensor("b2", [64, 1], F32, kind="ExternalInput").ap()
    b3t = nc.dram_tensor("b3sb", [128, 4], F32, kind="ExternalInput").ap()
    fbt = nc.dram_tensor("fc2b", [1, 10], F32, kind="ExternalInput").ap()
    ot = nc.dram_tensor("out", [BPC, 10], F32, kind="ExternalOutput").ap()

    from contextlib import ExitStack

    with tile.TileContext(nc, num_cores=NCORES) as tc, ExitStack() as es:
        W = es.enter_context(tc.tile_pool(name="weights", bufs=1))
        S = es.enter_context(tc.tile_pool(name="work", bufs=2))
        P = es.enter_context(tc.tile_pool(name="persist", bufs=1))
        PS = es.enter_context(tc.tile_pool(name="ps", bufs=6, space="PSUM"))

        # ---- load weights ----
        wc1 = W.tile([100, 128], BF16)
        nc.sync.dma_start(out=wc1[:], in_=wc1t)
        w2a = W.tile([128, 320], BF16)
        nc.sync.dma_start(out=w2a[:], in_=w2at)
        w2b = W.tile([32, 320], BF16)
        nc.sync.dma_start(out=w2b[:], in_=w2bt)
        w3 = W.tile([128, 4096], BF16)
        nc.sync.dma_start(out=w3[:], in_=w3t)
        fc2 = W.tile([128, 40], F32)
        nc.sync.dma_start(out=fc2[:], in_=fc2t)
        b1r = W.tile([128, 1], F32)
        nc.sync.dma_start(out=b1r[:], in_=b1t)
        b2 = W.tile([64, 1], F32)
        nc.sync.dma_start(out=b2[:], in_=b2t)
        b3 = W.tile([128, 4], F32)
        nc.sync.dma_start(out=b3[:], in_=b3t)
        fc2b = W.tile([1, 10], F32)
        nc.sync.dma_start(out=fc2b[:], in_=fbt)
        ones1 = W.tile([1, 128], F32)
        nc.vector.memset(ones1[:], 1.0)

        h2 = P.tile([64, 16 * BPC], BF16)          # free = (sp outer, b inner)
        DR = es.enter_context(tc.tile_pool(name="dram", bufs=2, space="DRAM"))

        for blk in range(NBLK):
            xbase = blk * XBLK
            # ---- conv1 input: 2-hop shift-replication -> [100, CHUNKF] ----
            # hop A: partition 5j+kh = chunk j shifted by image-row kh
            # hop B: partition 25j+5kh+kw = hop-A partition shifted by kw
            xrep = S.tile([120, CHUNKF + 8], BF16, tag="xrep")
            srcA = bass.AP(
                tensor=xt.tensor,
                offset=xbase,
                ap=[[CHUNKF, 4], [28, 5], [1, CHUNKF]],
            )
            nc.sync.dma_start(out=xrep[100:120, 0:CHUNKF], in_=srcA)
            srcB = bass.AP(
                tensor=xrep[:].tensor,
                offset=xrep[:].offset + 100 * (CHUNKF + 8),
                ap=[[CHUNKF + 8, 20], [1, 5], [1, CHUNKF]],
            )
            nc.gpsimd.dma_start(out=xrep[0:100, 0:CHUNKF], in_=srcB)

            # ---- conv1 matmuls + pool (bf16 PSUM: one matmul per bank) ----
            h1p = S.tile([128, H1F], BF16, tag="h1p")
            for s in range(CS):
                for h in range(2):
                    ps1 = PS.tile([128, 288], F32, tag="ps")
                    rhs = bass.AP(
                        tensor=xrep[:].tensor,
                        offset=xrep[:].offset + s * 784 + h * 336,
                        ap=[[CHUNKF + 8, 100], [28, 12], [1, 24]],
                    )
                    nc.tensor.matmul(
                        out=ps1[:], lhsT=wc1[:], rhs=rhs, start=True, stop=True
                    )
                    pv = ps1[:].rearrange(
                        "p (ph s1 pw s0) -> p ph pw s1 s0", ph=6, s1=2, pw=12, s0=2
                    )
                    ov = bass.AP(
                        tensor=h1p[:].tensor,
                        offset=h1p[:].offset + s * 144 + h * 72,
                        ap=[[H1F, 128], [12, 6], [1, 12]],
                    )
                    nc.vector.tensor_reduce(out=ov, in_=pv, axis=AXY, op=MAXOP)
            # bias + relu (both commute with maxpool)
            nc.scalar.activation(h1p[:], h1p[:], RELU, bias=b1r[:])

            # ---- conv2 input: kw-shifted 4x replication via DRAM bounce ----
            # write h1p to DRAM permuted [ci, j, f]; read back so partition
            # 32c+ci holds (chunk j at free j*H1F) shifted by c
            h1d = DR.tile([32 * 4 * H1F + 8], BF16, tag="h1d")
            for j in range(4):
                dst = bass.AP(
                    tensor=h1d[:].tensor,
                    offset=h1d[:].offset + j * H1F,
                    ap=[[4 * H1F, 32], [1, H1F]],
                )
                eng = nc.sync if j % 2 == 0 else nc.gpsimd
                eng.dma_start(out=dst, in_=h1p[32 * j : 32 * j + 32, :])
            h1r = S.tile([128, 4 * H1F + 8], BF16, tag="h1r")
            rdsrc = bass.AP(
                tensor=h1d[:].tensor,
                offset=h1d[:].offset,
                ap=[[1, 4], [4 * H1F, 32], [1, 4 * H1F]],
            )
            nc.scalar.dma_start(out=h1r[0:128, 0 : 4 * H1F], in_=rdsrc)

            for g in range(8):            # 8 sample-groups of 8 within block
                j, hh = divmod(g, 2)
                goff = j * H1F + hh * 8 * 144
                ps2 = PS.tile([64, 512], F32, tag="ps")
                for kh in range(5):
                    rhs = bass.AP(
                        tensor=h1r[:].tensor,
                        offset=h1r[:].offset + goff + 12 * kh,
                        ap=[[4 * H1F + 8, 128], [144, 8], [12, 8], [1, 8]],
                    )
                    nc.tensor.matmul(
                        out=ps2[:],
                        lhsT=w2a[:, 64 * kh : 64 * kh + 64],
                        rhs=rhs,
                        start=(kh == 0),
                        stop=False,
                    )
                for kh in range(5):
                    rhs = bass.AP(
                        tensor=h1r[:].tensor,
                        offset=h1r[:].offset + goff + 12 * kh + 4,
                        ap=[[4 * H1F + 8, 32], [144, 8], [12, 8], [1, 8]],
                    )
                    nc.tensor.matmul(
                        out=ps2[:],
                        lhsT=w2b[:, 64 * kh : 64 * kh + 64],
                        rhs=rhs,
                        start=False,
                        stop=(kh == 4),
                    )
                # pool conv2 8x8 -> 4x4 in two stages
                st1 = S.tile([64, 256], F32, tag="st1")
                iv = ps2[:].rearrange("p (soh pw s0) -> p soh pw s0", pw=4, s0=2)
                nc.vector.tensor_reduce(out=st1[:], in_=iv, axis=AX, op=MAXOP)
                # st1 free = (s, oh, pw): flat = s*32 + (2ph+s1)*4 + pw
                b0 = blk * BLK + g * 8
                outv = bass.AP(
                    tensor=h2[:].tensor,
                    offset=h2[:].offset + b0,
                    ap=[[16 * BPC, 64], [4 * BPC, 4], [BPC, 4], [1, 8]],
                )
                ia = bass.AP(
                    tensor=st1[:].tensor,
                    offset=st1[:].offset,
                    ap=[[256, 64], [8, 4], [1, 4], [32, 8]],
                )
                ib = bass.AP(
                    tensor=st1[:].tensor,
                    offset=st1[:].offset + 4,
                    ap=[[256, 64], [8, 4], [1, 4], [32, 8]],
                )
                nc.vector.tensor_tensor(out=outv, in0=ia, in1=ib, op=MAXOP)

        # ---- h2 bias + relu ----
        nc.scalar.activation(h2[:], h2[:], RELU, bias=b2[:])

        # ---- fc1: relayout h2 -> 8 K-chunks [128, BPC] ----
        hr = []
        for k in range(8):
            t = P.tile([128, BPC + 8], BF16, tag=f"hr{k}")
            src = bass.AP(
                tensor=h2[:].tensor,
                offset=h2[:].offset + 8 * k * 16 * BPC,
                ap=[[16 * BPC, 8], [BPC, 16], [1, BPC]],
            )
            nc.sync.dma_start(out=t[:, 0:BPC], in_=src)
            hr.append(t)

        h3 = []
        for m in range(4):
            psf = PS.tile([128, 512], F32, tag="ps")
            for k in range(8):
                nc.tensor.matmul(
                    out=psf[:],
                    lhsT=w3[:, (k * 4 + m) * 128 : (k * 4 + m) * 128 + 128],
                    rhs=hr[k][:, 0:BPC],
                    start=(k == 0),
                    stop=(k == 7),
                )
            t = P.tile([128, BPC], F32, tag=f"h3{m}")
            nc.scalar.activation(t[:], psf[:], RELU, bias=b3[:, m : m + 1])
            h3.append(t)

        # ---- fc2 + log_softmax, batch on partitions ----
        for bc in range(4):
            psl = PS.tile([128, 10], F32, tag="ps")
            for k in range(4):
                nc.tensor.matmul(
                    out=psl[:],
                    lhsT=h3[k][:, bc * 128 : bc * 128 + 128],
                    rhs=fc2[:, k * 10 : k * 10 + 10],
                    start=(k == 0),
                    stop=False,
                )
            nc.tensor.matmul(
                out=psl[:],
                lhsT=ones1[:],
                rhs=fc2b[:],
                start=False,
                stop=True,
            )
            negm = S.tile([128, 1], F32, tag="negm")
            nc.vector.tensor_reduce(
                out=negm[:], in_=psl[:], axis=AX, op=MAXOP, negate=True
            )
            shifted = S.tile([128, 10], F32, tag="shifted")
            nc.vector.tensor_scalar(
                out=shifted[:], in0=psl[:], scalar1=negm[:], scalar2=None, op0=ADDOP
            )
            ex = S.tile([128, 10], F32, tag="ex")
            se = S.tile([128, 1], F32, tag="se")
            nc.scalar.activation(ex[:], shifted[:], EXP, accum_out=se[:])
            lse = S.tile([128, 1], F32, tag="lse")
            nc.scalar.activation(lse[:], se[:], LN)
            osb = S.tile([128, 10], F32, tag="osb")
            nc.vector.tensor_scalar(
                out=osb[:], in0=shifted[:], scalar1=lse[:], scalar2=None, op0=SUBOP
            )
            nc.sync.dma_start(out=ot[bc * 128 : bc * 128 + 128, :], in_=osb[:])

    nc.finalize()
    return nc


def _prep_weights(inputs):
    """Host-side: densify sketch weights and lay them out for the kernel."""
    h1, h2i, h3i = inputs["hash_idx1"], inputs["hash_idx2"], inputs["hash_idx3"]
    s1, s2, s3 = inputs["sgn1"], inputs["sgn2"], inputs["sgn3"]
    w1, w2, w3 = inputs["w1"], inputs["w2"], inputs["w3"]
    b1, b2, b3 = inputs["b1"], inputs["b2"], inputs["b3"]
    fc2w, fc2b = inputs["fc2_w"], inputs["fc2_b"]

    wc1 = (w1[:, h1] * s1[None, :]).astype(np.float32)            # (32, 25)
    wc2 = (w2[:, h2i] * s2[None, :]).astype(np.float32).reshape(64, 32, 5, 5)
    W3 = (w3[:, h3i] * s3[None, :]).astype(np.float32)            # (512, 1024)

    wc1bd = np.zeros((100, 128), np.float32)
    for j in range(4):
        wc1bd[25 * j : 25 * j + 25, 32 * j : 32 * j + 32] = wc1.T
    # conv2 pass A: lhsT rows (c=kw-copy, ci), cols co; tap (kh, kw=c)
    w2a = np.zeros((128, 5, 64), np.float32)
    for c in range(4):
        for kh in range(5):
            w2a[32 * c : 32 * c + 32, kh, :] = wc2[:, :, kh, c].T
    w2a = w2a.reshape(128, 320)
    # conv2 pass B: kw=4 taps via copy 0
    w2b = np.zeros((32, 5, 64), np.float32)
    for kh in range(5):
        w2b[:, kh, :] = wc2[:, :, kh, 4].T
    w2b = w2b.reshape(32, 320)

    # fc1: lhsT chunk (k,m) = W3.T[128k:128k+128, 128m:128m+128]
    w3sb = np.zeros((128, 8, 4, 128), np.float32)
    W3T = np.ascontiguousarray(W3.T)  # (1024, 512)
    for k in range(8):
        for m in range(4):
            w3sb[:, k, m, :] = W3T[128 * k : 128 * k + 128, 128 * m : 128 * m + 128]
    w3sb = w3sb.reshape(128, 4096)

    fc2sb = np.zeros((128, 4, 10), np.float32)
    for k in range(4):
        fc2sb[:, k, :] = fc2w[:, 128 * k : 128 * k + 128].T
    fc2sb = fc2sb.reshape(128, 40)

    b1r = np.tile(np.asarray(b1, np.float32), 4).reshape(128, 1)
    b3sb = np.asarray(b3, np.float32).reshape(4, 128).T.copy()

    bf = lambda a: np.asarray(a, dtype=ml_dtypes.bfloat16)
    f = lambda a: np.ascontiguousarray(a, dtype=np.float32)
    return {
        "wc1bd": bf(wc1bd),
        "w2a": bf(w2a),
        "w2b": bf(w2b),
        "w3sb": bf(w3sb),
        "fc2sb": f(fc2sb),
        "b1r": f(b1r),
        "b2": f(np.asarray(b2).reshape(64, 1)),
        "b3sb": f(b3sb),
        "fc2b": f(np.asarray(fc2b).reshape(1, 10)),
    }


def kernel(**inputs):
    out, _ = _run(inputs, trace=False)
    return out


def _run(inputs, trace=False):
    if "nc" not in _CACHE:
        _CACHE["nc"] = _build()
    nc = _CACHE["nc"]

    wmap = _prep_weights(inputs)
    x = np.asarray(inputs["x"], np.float32).reshape(4096, 784)

    in_maps = []
    for c in range(NCORES):
        xs = x[c * BPC : (c + 1) * BPC].reshape(-1)
        xs = np.concatenate([xs, np.zeros(XPAD, np.float32)])
        m = dict(wmap)
        m["x"] = np.asarray(xs, dtype=ml_dtypes.bfloat16)
        in_maps.append(m)

    res = run_bass_kernel_spmd(
        nc, in_maps, core_ids=list(range(NCORES)), trace=trace
    )
    out = np.concatenate([res.results[c]["out"] for c in range(NCORES)], axis=0)
    return out.astype(np.float32), res



# revision 8
# speedup vs baseline: 1.2034x; 1.2034x over previous
"""Trainium2 Bass kernel for nn_CNNMnist_Sketch (sketched CNN forward pass).

Data-parallel over 8 NeuronCores: batch 4096 -> 512 per core.
Per-core pipeline (all shapes hardcoded):
  conv1 5x5 (1->32ch) + maxpool2 + relu   -> h1  [32ch, 12x12]
  conv2 5x5 (32->64ch) + maxpool2 + relu  -> h2  [64ch, 4x4] -> flat 1024
  fc1 1024->512 + relu, fc2 512->10, log_softmax

Layout/scheduling notes:
  - conv1: input replicated to 100 SBUF partitions (4 batch-chunks x 25 taps),
    each partition pre-shifted by its tap offset; a single block-diagonal
    [100,128] lhsT computes 4 chunks x 32 channels in one matmul stream.
  - conv2: pooled h1 bounced through DRAM and read back twice: h1r holds 4
    kw-shifted copies (shift 0..3), h1r2 holds 4 kh-shifted copies (shift
    0,12,24,36).  25 taps then contract in 7 passes (6 full-K + 1 K=32)
    instead of 10.  conv2 lhsT output channels are duplicated to M=128 so
    the PE activity monitor keeps the array at full clock (M=64 matmuls
    never unthrottle HAM from 1.2 to 2.4 GHz).
  - software pipelining: tensor queue order is conv1(k+1); conv2(k), so the
    relu/bounce/read chain for block k+1 hides under conv2(k)'s matmuls.
  - h2 lives in DRAM (feature-major) so fc1's k-chunk reads are contiguous.
"""

import numpy as np
import ml_dtypes

import concourse.bass as bass
import concourse.bacc as bacc
import concourse.tile as tile
from concourse import mybir
from concourse.bass_utils import run_bass_kernel_spmd

F32 = mybir.dt.float32
BF16 = mybir.dt.bfloat16
RELU = mybir.ActivationFunctionType.Relu
EXP = mybir.ActivationFunctionType.Exp
LN = mybir.ActivationFunctionType.Ln
MAXOP = mybir.AluOpType.max
SUBOP = mybir.AluOpType.subtract
ADDOP = mybir.AluOpType.add
AXY = mybir.AxisListType.XY
AX = mybir.AxisListType.X

NCORES = 8
BPC = 4096 // NCORES          # samples per core
BLK = 64                      # samples per block
NBLK = BPC // BLK
CS = BLK // 4                 # samples per conv1 chunk (4 chunks / block)
CHUNKF = CS * 784             # x elements per chunk
XBLK = BLK * 784              # x elements per block
H1F = CS * 144                # h1 elements per chunk (per channel)
XPAD = 128                    # DRAM pad so shifted reads never go OOB
HEAT = 56                     # warmup matmuls to unthrottle the PE clock

_CACHE = {}


def _build():
    nc = bacc.Bacc(target_bir_lowering=False, debug=False, num_devices=NCORES)

    xt = nc.dram_tensor("x", [BPC * 784 + XPAD], BF16, kind="ExternalInput").ap()
    wc1t = nc.dram_tensor("wc1bd", [100, 128], BF16, kind="ExternalInput").ap()
    w2at = nc.dram_tensor("w2a5", [128, 5 * 128], BF16, kind="ExternalInput").ap()
    w2bt = nc.dram_tensor("w2b1", [128, 128], BF16, kind="ExternalInput").ap()
    w2ct = nc.dram_tensor("w2c", [32, 128], BF16, kind="ExternalInput").ap()
    w3t = nc.dram_tensor("w3sb", [128, 4096], BF16, kind="ExternalInput").ap()
    fc2t = nc.dram_tensor("fc2sb", [128, 40], F32, kind="ExternalInput").ap()
    b1t = nc.dram_tensor("b1r", [128, 1], F32, kind="ExternalInput").ap()
    b2t = nc.dram_tensor("b2", [64, 1], F32, kind="ExternalInput").ap()
    b3t = nc.dram_tensor("b3sb", [128, 4], F32, kind="ExternalInput").ap()
    fbt = nc.dram_tensor("fc2b", [1, 10], F32, kind="ExternalInput").ap()
    ot = nc.dram_tensor("out", [BPC, 10], F32, kind="ExternalOutput").ap()

    from contextlib import ExitStack

    with tile.TileContext(nc, num_cores=NCORES) as tc, ExitStack() as es:
        W = es.enter_context(tc.tile_pool(name="weights", bufs=1))
        S = es.enter_context(tc.tile_pool(name="work", bufs=2))
        P = es.enter_context(tc.tile_pool(name="persist", bufs=1))
        PS = es.enter_context(tc.tile_pool(name="ps", bufs=8, space="PSUM"))
        DR = es.enter_context(tc.tile_pool(name="dram", bufs=2, space="DRAM"))
        DR2 = es.enter_context(tc.tile_pool(name="dram2", bufs=1, space="DRAM"))

        # ---- load weights ----
        wc1 = W.tile([100, 128], BF16)
        nc.sync.dma_start(out=wc1[:], in_=wc1t)
        w2a = W.tile([128, 640], BF16)
        nc.sync.dma_start(out=w2a[:], in_=w2at)
        w2b = W.tile([128, 128], BF16)
        nc.sync.dma_start(out=w2b[:], in_=w2bt)
        w2c = W.tile([32, 128], BF16)
        nc.sync.dma_start(out=w2c[:], in_=w2ct)
        w3 = W.tile([128, 4096], BF16)
        nc.sync.dma_start(out=w3[:], in_=w3t)
        fc2 = W.tile([128, 40], F32)
        nc.sync.dma_start(out=fc2[:], in_=fc2t)
        b1r = W.tile([128, 1], F32)
        nc.sync.dma_start(out=b1r[:], in_=b1t)
        b2 = W.tile([64, 1], F32)
        nc.sync.dma_start(out=b2[:], in_=b2t)
        b3 = W.tile([128, 4], F32)
        nc.sync.dma_start(out=b3[:], in_=b3t)
        fc2b = W.tile([1, 10], F32)
        nc.sync.dma_start(out=fc2b[:], in_=fbt)
        ones1 = W.tile([1, 128], F32)
        nc.vector.memset(ones1[:], 1.0)
        heat = W.tile([128, 512], BF16)
        nc.vector.memset(heat[:], 1.0)

        h2d = DR2.tile([1024 * BPC], BF16, tag="h2d")

        def load_xrep(blk):
            """2-hop shift-replication of block's x -> xrep [100, CHUNKF].
            hop A: partition 100+5j+kh = chunk j shifted by image-row kh
            hop B: partition 25j+5kh+kw = hop-A partition shifted by kw."""
            xbase = blk * XBLK
            xrep = S.tile([120, CHUNKF + 8], BF16, tag="xrep")
            srcA = bass.AP(
                tensor=xt.tensor,
                offset=xbase,
                ap=[[CHUNKF, 4], [28, 5], [1, CHUNKF]],
            )
            nc.sync.dma_start(out=xrep[100:120, 0:CHUNKF], in_=srcA)
            for half, eng in ((0, nc.gpsimd), (1, nc.scalar)):
                srcB = bass.AP(
                    tensor=xrep[:].tensor,
                    offset=xrep[:].offset + (100 + 10 * half) * (CHUNKF + 8),
                    ap=[[CHUNKF + 8, 10], [1, 5], [1, CHUNKF]],
                )
                eng.dma_start(
                    out=xrep[50 * half : 50 * half + 50, 0:CHUNKF], in_=srcB
                )
            return xrep

        def conv1(xrep):
            """conv1 matmuls + pool; returns relu'd h1p [128=(j,co), H1F]."""
            h1p = S.tile([128, H1F], BF16, tag="h1p")
            for s in range(CS):
                for h in range(2):
                    ps1 = PS.tile([128, 512], F32, tag="ps")
                    rhs = bass.AP(
                        tensor=xrep[:].tensor,
                        offset=xrep[:].offset + s * 784 + h * 336,
                        ap=[[CHUNKF + 8, 100], [28, 12], [1, 24]],
                    )
                    nc.tensor.matmul(
                        out=ps1[:, 0:288], lhsT=wc1[:], rhs=rhs,
                        start=True, stop=True,
                    )
                    pv = ps1[:, 0:288].rearrange(
                        "p (ph s1 pw s0) -> p ph pw s1 s0", ph=6, s1=2, pw=12, s0=2
                    )
                    ov = bass.AP(
                        tensor=h1p[:].tensor,
                        offset=h1p[:].offset + s * 144 + h * 72,
                        ap=[[H1F, 128], [12, 6], [1, 12]],
                    )
                    nc.vector.tensor_reduce(out=ov, in_=pv, axis=AXY, op=MAXOP)
            # bias + relu per j-quarter so bounce writes can start early
            for j in range(4):
                nc.scalar.activation(
                    h1p[32 * j : 32 * j + 32, :],
                    h1p[32 * j : 32 * j + 32, :],
                    RELU,
                    bias=b1r[32 * j : 32 * j + 32, :],
                )
            return h1p

        def bounce_and_read(h1p):
            """DRAM bounce: h1d layout [ci][j][f], then two 4-copy reads:
            h1r partition (c,ci) shift c in {0,1,2,3} (kw shifts),
            h1r2 partition (c2,ci) shift 12*c2 (kh shifts)."""
            h1d = DR.tile([32 * 4 * H1F + 64], BF16, tag="h1d")
            for j in range(4):
                dst = bass.AP(
                    tensor=h1d[:].tensor,
                    offset=h1d[:].offset + j * H1F,
                    ap=[[4 * H1F, 32], [1, H1F]],
                )
                eng = nc.gpsimd if j < 2 else nc.sync
                eng.dma_start(out=dst, in_=h1p[32 * j : 32 * j + 32, :])
            h1r = S.tile([128, 4 * H1F + 8], BF16, tag="h1r")
            h1r2 = S.tile([128, 4 * H1F + 8], BF16, tag="h1r2")
            for half in range(2):
                fo = half * 2 * H1F
                rd = bass.AP(
                    tensor=h1d[:].tensor,
                    offset=h1d[:].offset + fo,
                    ap=[[1, 4], [4 * H1F, 32], [1, 2 * H1F]],
                )
                nc.gpsimd.dma_start(out=h1r[:, fo : fo + 2 * H1F], in_=rd)
                rd2 = bass.AP(
                    tensor=h1d[:].tensor,
                    offset=h1d[:].offset + fo,
                    ap=[[12, 4], [4 * H1F, 32], [1, 2 * H1F]],
                )
                nc.sync.dma_start(out=h1r2[:, fo : fo + 2 * H1F], in_=rd2)
            return h1r, h1r2

        def conv2(blk, h1r, h1r2):
            """7-pass conv2 (pass-major over 4-group supergroups) + pool;
            writes relu'd pooled output to h2d DRAM (feature-major)."""
            h2s = S.tile([64, 1024], BF16, tag="h2s")
            for sg in range(2):            # j-pairs {0,1}, {2,3}
                groups = [(2 * sg + j, hh) for j in range(2) for hh in range(2)]
                banks = [
                    PS.tile([128, 512], F32, tag="ps", name=f"bank{gi}")
                    for gi in range(len(groups))
                ]
                # passes 0-4: lhsT (c,ci)->co for tap (kh, kw=c), offset 12kh
                # pass 5: lhsT (c2,ci)->co for tap (kh=c2, kw=4), offset 4
                # pass 6: K=32 c=0 copy, tap (4,4), offset 52
                passes = [
                    (h1r, 128, 12 * kh, w2a[:, 128 * kh : 128 * kh + 128])
                    for kh in range(5)
                ]
                passes.append((h1r2, 128, 4, w2b[:]))
                passes.append((h1r, 32, 52, w2c[:]))
                for p, (src, kk, off, lhsT) in enumerate(passes):
                    for gi, (j, hh) in enumerate(groups):
                        goff = j * H1F + hh * 8 * 144
                        rhs = bass.AP(
                            tensor=src[:].tensor,
                            offset=src[:].offset + goff + off,
                            ap=[[4 * H1F + 8, kk], [144, 8], [12, 8], [1, 8]],
                        )
                        nc.tensor.matmul(
                            out=banks[gi][:],
                            lhsT=lhsT,
                            rhs=rhs,
                            start=(p == 0),
                            stop=(p == 6),
                        )
                # pool conv2 8x8 -> 4x4 in two stages per group
                for gi, (j, hh) in enumerate(groups):
                    g = 2 * j + hh
                    st1 = S.tile([64, 256], F32, tag="st1")
                    iv = banks[gi][0:64, :].rearrange(
                        "p (soh pw s0) -> p soh pw s0", pw=4, s0=2
                    )
                    nc.vector.tensor_reduce(out=st1[:], in_=iv, axis=AX, op=MAXOP)
                    outv = bass.AP(
                        tensor=h2s[:].tensor,
                        offset=h2s[:].offset + g * 8,
                        ap=[[1024, 64], [256, 4], [64, 4], [1, 8]],
                    )
                    ia = bass.AP(
                        tensor=st1[:].tensor,
                        offset=st1[:].offset,
                        ap=[[256, 64], [8, 4], [1, 4], [32, 8]],
                    )
                    ib = bass.AP(
                        tensor=st1[:].tensor,
                        offset=st1[:].offset + 4,
                        ap=[[256, 64], [8, 4], [1, 4], [32, 8]],
                    )
                    nc.vector.tensor_tensor(out=outv, in0=ia, in1=ib, op=MAXOP)
            nc.scalar.activation(h2s[:], h2s[:], RELU, bias=b2[:])
            dst = bass.AP(
                tensor=h2d[:].tensor,
                offset=h2d[:].offset + blk * 64,
                ap=[[16 * BPC, 64], [BPC, 16], [1, 64]],
            )
            src = bass.AP(
                tensor=h2s[:].tensor,
                offset=h2s[:].offset,
                ap=[[1024, 64], [64, 16], [1, 64]],
            )
            nc.scalar.dma_start(out=dst, in_=src)

        # ---- prologue: prefetch x for blocks 0 and 1, warm up the PE ----
        xreps = {0: load_xrep(0), 1: load_xrep(1)}
        for _ in range(HEAT):
            psh = PS.tile([128, 512], F32, tag="ps")
            nc.tensor.matmul(
                out=psh[:], lhsT=heat[:, 0:128], rhs=heat[:], start=True, stop=True
            )
        h1p0 = conv1(xreps.pop(0))
        reads = {0: bounce_and_read(h1p0)}

        # ---- pipelined block loop: conv1(k+1) then conv2(k) ----
        for k in range(NBLK):
            if k + 1 < NBLK:
                h1p = conv1(xreps.pop(k + 1))
                reads[k + 1] = bounce_and_read(h1p)
            if k + 2 < NBLK:
                xreps[k + 2] = load_xrep(k + 2)
            h1r, h1r2 = reads.pop(k)
            conv2(k, h1r, h1r2)

        # ---- fc1: 8 contiguous k-chunk reads from h2d ----
        hr = []
        engs = [nc.sync, nc.gpsimd, nc.scalar]
        for kch in range(8):
            t = P.tile([128, BPC], BF16, tag=f"hr{kch}")
            src = bass.AP(
                tensor=h2d[:].tensor,
                offset=h2d[:].offset + kch * 128 * BPC,
                ap=[[BPC, 128], [1, BPC]],
            )
            engs[kch % 3].dma_start(out=t[:], in_=src)
            hr.append(t)

        h3 = []
        for m in range(4):
            psf = PS.tile([128, 512], F32, tag="ps")
            for kch in range(8):
                nc.tensor.matmul(
                    out=psf[:],
                    lhsT=w3[:, (kch * 4 + m) * 128 : (kch * 4 + m) * 128 + 128],
                    rhs=hr[kch][:],
                    start=(kch == 0),
                    stop=(kch == 7),
                )
            t = P.tile([128, BPC], F32, tag=f"h3{m}")
            nc.scalar.activation(t[:], psf[:], RELU, bias=b3[:, m : m + 1])
            h3.append(t)

        # ---- fc2 + log_softmax, batch on partitions ----
        for bc in range(4):
            psl = PS.tile([128, 10], F32, tag="ps")
            for kch in range(4):
                nc.tensor.matmul(
                    out=psl[:],
                    lhsT=h3[kch][:, bc * 128 : bc * 128 + 128],
                    rhs=fc2[:, kch * 10 : kch * 10 + 10],
                    start=(kch == 0),
                    stop=False,
                )
            nc.tensor.matmul(
                out=psl[:],
                lhsT=ones1[:],
                rhs=fc2b[:],
                start=False,
                stop=True,
            )
            negm = S.tile([128, 1], F32, tag="negm")
            nc.vector.tensor_reduce(
                out=negm[:], in_=psl[:], axis=AX, op=MAXOP, negate=True
            )
            shifted = S.tile([128, 10], F32, tag="shifted")
            nc.vector.tensor_scalar(
                out=shifted[:], in0=psl[:], scalar1=negm[:], scalar2=None, op0=ADDOP
            )
            ex = S.tile([128, 10], F32, tag="ex")
            se = S.tile([128, 1], F32, tag="se")
            nc.scalar.activation(ex[:], shifted[:], EXP, accum_out=se[:])
            lse = S.tile([128, 1], F32, tag="lse")
            nc.scalar.activation(lse[:], se[:], LN)
            osb = S.tile([128, 10], F32, tag="osb")
            nc.vector.tensor_scalar(
                out=osb[:], in0=shifted[:], scalar1=lse[:], scalar2=None, op0=SUBOP
            )
            nc.sync.dma_start(out=ot[bc * 128 : bc * 128 + 128, :], in_=osb[:])

    nc.finalize()
    return nc


def _prep_weights(inputs):
    """Host-side: densify sketch weights and lay them out for the kernel."""
    h1, h2i, h3i = inputs["hash_idx1"], inputs["hash_idx2"], inputs["hash_idx3"]
    s1, s2, s3 = inputs["sgn1"], inputs["sgn2"], inputs["sgn3"]
    w1, w2, w3 = inputs["w1"], inputs["w2"], inputs["w3"]
    b1, b2, b3 = inputs["b1"], inputs["b2"], inputs["b3"]
    fc2w, fc2b = inputs["fc2_w"], inputs["fc2_b"]

    wc1 = (w1[:, h1] * s1[None, :]).astype(np.float32)            # (32, 25)
    wc2 = (w2[:, h2i] * s2[None, :]).astype(np.float32).reshape(64, 32, 5, 5)
    W3 = (w3[:, h3i] * s3[None, :]).astype(np.float32)            # (512, 1024)

    wc1bd = np.zeros((100, 128), np.float32)
    for j in range(4):
        wc1bd[25 * j : 25 * j + 25, 32 * j : 32 * j + 32] = wc1.T

    # conv2 lhsT packs; output channels duplicated to M=128 (HAM heater)
    w2a5 = np.zeros((128, 5, 64), np.float32)
    for c in range(4):
        for kh in range(5):
            w2a5[32 * c : 32 * c + 32, kh, :] = wc2[:, :, kh, c].T
    w2a5 = np.concatenate([w2a5, w2a5], axis=2).reshape(128, 640)
    w2b1 = np.zeros((128, 64), np.float32)
    for c2 in range(4):
        w2b1[32 * c2 : 32 * c2 + 32, :] = wc2[:, :, c2, 4].T
    w2b1 = np.concatenate([w2b1, w2b1], axis=1)
    w2c = np.concatenate([wc2[:, :, 4, 4].T, wc2[:, :, 4, 4].T], axis=1)

    # fc1: lhsT chunk (k,m) = W3.T[128k:128k+128, 128m:128m+128]
    w3sb = np.zeros((128, 8, 4, 128), np.float32)
    W3T = np.ascontiguousarray(W3.T)  # (1024, 512)
    for k in range(8):
        for m in range(4):
            w3sb[:, k, m, :] = W3T[128 * k : 128 * k + 128, 128 * m : 128 * m + 128]
    w3sb = w3sb.reshape(128, 4096)

    fc2sb = np.zeros((128, 4, 10), np.float32)
    for k in range(4):
        fc2sb[:, k, :] = fc2w[:, 128 * k : 128 * k + 128].T
    fc2sb = fc2sb.reshape(128, 40)

    b1r = np.tile(np.asarray(b1, np.float32), 4).reshape(128, 1)
    b3sb = np.asarray(b3, np.float32).reshape(4, 128).T.copy()

    bf = lambda a: np.asarray(a, dtype=ml_dtypes.bfloat16)
    f = lambda a: np.ascontiguousarray(a, dtype=np.float32)
    return {
        "wc1bd": bf(wc1bd),
        "w2a5": bf(w2a5),
        "w2b1": bf(w2b1),
        "w2c": bf(w2c),
        "w3sb": bf(w3sb),
        "fc2sb": f(fc2sb),
        "b1r": f(b1r),
        "b2": f(np.asarray(b2).reshape(64, 1)),
        "b3sb": f(b3sb),
        "fc2b": f(np.asarray(fc2b).reshape(1, 10)),
    }


def kernel(**inputs):
    out, _ = _run(inputs, trace=False)
    return out


def _run(inputs, trace=False):
    if "nc" not in _CACHE:
        _CACHE["nc"] = _build()
    nc = _CACHE["nc"]

    wmap = _prep_weights(inputs)
    x = np.asarray(inputs["x"], np.float32).reshape(4096, 784)

    in_maps = []
    for c in range(NCORES):
        xs = x[c * BPC : (c + 1) * BPC].reshape(-1)
        xs = np.concatenate([xs, np.zeros(XPAD, np.float32)])
        m = dict(wmap)
        m["x"] = np.asarray(xs, dtype=ml_dtypes.bfloat16)
        in_maps.append(m)

    res = run_bass_kernel_spmd(
        nc, in_maps, core_ids=list(range(NCORES)), trace=trace
    )
    out = np.concatenate([res.results[c]["out"] for c in range(NCORES)], axis=0)
    return out.astype(np.float32), res


# revision 18
# speedup vs baseline: 2.0155x; 1.6749x over previous
"""Trainium2 Bass kernel for nn_CNNMnist_Sketch (sketched CNN forward pass).

Data-parallel over 8 NeuronCores: batch 4096 -> 512 per core.
Per-core pipeline (all shapes hardcoded):
  conv1 5x5 (1->32ch) + maxpool2 + relu   -> h1  [32ch, 12x12]
  conv2 5x5 (32->64ch) + maxpool2 + relu  -> h2  [64ch, 4x4] -> flat 1024
  fc1 1024->512 + relu, fc2 512->10, log_softmax

Layout/scheduling notes:
  - conv1: input replicated to 100 SBUF partitions (4 batch-chunks x 25 taps),
    each partition pre-shifted by its tap offset; a single block-diagonal
    [100,128] lhsT computes 4 chunks x 32 channels in one matmul stream.
  - conv2: pooled h1 bounced through DRAM and read back twice: h1r holds 4
    kw-shifted copies (shift 0..3), h1r2 holds 4 kh-shifted copies (shift
    0,12,24,36).  25 taps then contract in 7 passes (6 full-K + 1 K=32)
    instead of 10.  conv2 lhsT output channels are duplicated to M=128 so
    the PE activity monitor keeps the array at full clock (M=64 matmuls
    never unthrottle HAM from 1.2 to 2.4 GHz).
  - software pipelining: tensor queue order is conv1(k+1); conv2(k), so the
    relu/bounce/read chain for block k+1 hides under conv2(k)'s matmuls.
  - h2 lives in DRAM (feature-major) so fc1's k-chunk reads are contiguous.
"""

import numpy as np
import ml_dtypes

import concourse.bass as bass
import concourse.bacc as bacc
import concourse.tile as tile
from concourse import mybir
from concourse.bass_utils import run_bass_kernel_spmd

F32 = mybir.dt.float32
BF16 = mybir.dt.bfloat16
RELU = mybir.ActivationFunctionType.Relu
EXP = mybir.ActivationFunctionType.Exp
LN = mybir.ActivationFunctionType.Ln
MAXOP = mybir.AluOpType.max
SUBOP = mybir.AluOpType.subtract
ADDOP = mybir.AluOpType.add
AXY = mybir.AxisListType.XY
AX = mybir.AxisListType.X

NCORES = 8
BPC = 4096 // NCORES          # samples per core
BLK = 64                      # samples per block
NBLK = BPC // BLK
CS = BLK // 4                 # samples per conv1 chunk (4 chunks / block)
CHUNKF = CS * 784             # x elements per chunk
XBLK = BLK * 784              # x elements per block
H1F = CS * 144                # h1 elements per chunk (per channel)
XPAD = 128                    # DRAM pad so shifted reads never go OOB
HEAT = 56                     # warmup matmuls to unthrottle the PE clock

_CACHE = {}


def _build():
    nc = bacc.Bacc(target_bir_lowering=False, debug=False, num_devices=NCORES)

    xt = nc.dram_tensor("x", [BPC * 784 + XPAD], BF16, kind="ExternalInput").ap()
    wc1t = nc.dram_tensor("wc1bd", [100, 128], BF16, kind="ExternalInput").ap()
    w2at = nc.dram_tensor("w2a5", [128, 5 * 128], BF16, kind="ExternalInput").ap()
    w2bt = nc.dram_tensor("w2b5", [32, 5 * 128], BF16, kind="ExternalInput").ap()
    w3t = nc.dram_tensor("w3sb", [128, 4096], BF16, kind="ExternalInput").ap()
    fc2t = nc.dram_tensor("fc2sb", [128, 40], F32, kind="ExternalInput").ap()
    b1t = nc.dram_tensor("b1r", [128, 1], F32, kind="ExternalInput").ap()
    b2t = nc.dram_tensor("b2", [64, 1], F32, kind="ExternalInput").ap()
    b3t = nc.dram_tensor("b3sb", [128, 4], F32, kind="ExternalInput").ap()
    fbt = nc.dram_tensor("fc2b", [1, 10], F32, kind="ExternalInput").ap()
    ot = nc.dram_tensor("out", [BPC, 10], F32, kind="ExternalOutput").ap()

    from contextlib import ExitStack

    with tile.TileContext(nc, num_cores=NCORES) as tc, ExitStack() as es:
        W = es.enter_context(tc.tile_pool(name="weights", bufs=1))
        S = es.enter_context(tc.tile_pool(name="work", bufs=2))
        P = es.enter_context(tc.tile_pool(name="persist", bufs=1))
        PS = es.enter_context(tc.tile_pool(name="ps", bufs=8, space="PSUM"))
        DR2 = es.enter_context(tc.tile_pool(name="dram2", bufs=1, space="DRAM"))

        # ---- load weights ----
        wc1 = W.tile([100, 128], BF16)
        nc.sync.dma_start(out=wc1[:], in_=wc1t)
        w2a = W.tile([128, 640], BF16)
        nc.sync.dma_start(out=w2a[:], in_=w2at)
        w2b = W.tile([32, 640], BF16)
        nc.sync.dma_start(out=w2b[:], in_=w2bt)
        w3 = W.tile([128, 4096], BF16)
        nc.sync.dma_start(out=w3[:], in_=w3t)
        fc2 = W.tile([128, 40], F32)
        nc.sync.dma_start(out=fc2[:], in_=fc2t)
        b1r = W.tile([128, 1], F32)
        nc.sync.dma_start(out=b1r[:], in_=b1t)
        b2 = W.tile([64, 1], F32)
        nc.sync.dma_start(out=b2[:], in_=b2t)
        b3 = W.tile([128, 4], F32)
        nc.sync.dma_start(out=b3[:], in_=b3t)
        fc2b = W.tile([1, 10], F32)
        nc.sync.dma_start(out=fc2b[:], in_=fbt)
        ones1 = W.tile([1, 128], F32)
        nc.vector.memset(ones1[:], 1.0)
        heat = W.tile([128, 512], BF16)
        nc.vector.memset(heat[:], 1.0)

        h2d = DR2.tile([1024 * BPC], BF16, tag="h2d")

        def load_xrep(blk):
            """2-hop shift-replication of block's x -> xrep [100, CHUNKF].
            hop A: partition 100+5j+kh = chunk j shifted by image-row kh
            hop B: partition 25j+5kh+kw = hop-A partition shifted by kw."""
            xbase = blk * XBLK
            xrep = S.tile([120, CHUNKF + 8], BF16, tag="xrep")
            srcA = bass.AP(
                tensor=xt.tensor,
                offset=xbase,
                ap=[[CHUNKF, 4], [28, 5], [1, CHUNKF]],
            )
            nc.sync.dma_start(out=xrep[100:120, 0:CHUNKF], in_=srcA)
            for half, eng in ((0, nc.gpsimd), (1, nc.scalar)):
                srcB = bass.AP(
                    tensor=xrep[:].tensor,
                    offset=xrep[:].offset + (100 + 10 * half) * (CHUNKF + 8),
                    ap=[[CHUNKF + 8, 10], [1, 5], [1, CHUNKF]],
                )
                eng.dma_start(
                    out=xrep[50 * half : 50 * half + 50, 0:CHUNKF], in_=srcB
                )
            return xrep

        def conv1(xrep):
            """conv1 matmuls + pool; returns relu'd h1p [128=(j,co), H1F]."""
            h1p = S.tile([128, H1F + 8], BF16, tag="h1p")
            for s in range(CS):
                for h in range(2):
                    ps1 = PS.tile([128, 512], F32, tag="ps")
                    rhs = bass.AP(
                        tensor=xrep[:].tensor,
                        offset=xrep[:].offset + s * 784 + h * 336,
                        ap=[[CHUNKF + 8, 100], [28, 12], [1, 24]],
                    )
                    nc.tensor.matmul(
                        out=ps1[:, 0:288], lhsT=wc1[:], rhs=rhs,
                        start=True, stop=True,
                    )
                    pv = ps1[:, 0:288].rearrange(
                        "p (ph s1 pw s0) -> p ph pw s1 s0", ph=6, s1=2, pw=12, s0=2
                    )
                    ov = bass.AP(
                        tensor=h1p[:].tensor,
                        offset=h1p[:].offset + s * 144 + h * 72,
                        ap=[[H1F + 8, 128], [12, 6], [1, 12]],
                    )
                    nc.vector.tensor_reduce(out=ov, in_=pv, axis=AXY, op=MAXOP)
            nc.scalar.activation(
                h1p[:, 0:H1F], h1p[:, 0:H1F], RELU, bias=b1r[:]
            )
            return h1p

        def build_h1r(h1p):
            """Direct SBUF->SBUF shift-replication: h1r partition (c,ci)
            holds h1[ci, j, f+c] at free (j, f) -- kw shift c in {0..3}.
            One DMA per c so the outer AP dim (32) spreads descriptors
            across all 16 SDMA engines."""
            h1r = S.tile([128, 4 * H1F + 8], BF16, tag="h1r")
            pp = H1F + 8
            for c in range(4):
                eng = nc.gpsimd if c < 2 else nc.sync
                for j in range(4):
                    src = bass.AP(
                        tensor=h1p[:].tensor,
                        offset=h1p[:].offset + 32 * j * pp + c,
                        ap=[[pp, 32], [1, H1F]],
                    )
                    eng.dma_start(
                        out=h1r[32 * c : 32 * c + 32, j * H1F : (j + 1) * H1F],
                        in_=src,
                    )
            return h1r

        def conv2(blk, h1r):
            """10-pass conv2 (pass-major over 4-group supergroups) + pool;
            writes relu'd pooled output to h2d DRAM (feature-major)."""
            h2s = S.tile([64, 1024], BF16, tag="h2s")
            for sg in range(2):            # j-pairs {0,1}, {2,3}
                groups = [(2 * sg + j, hh) for j in range(2) for hh in range(2)]
                banks = [
                    PS.tile([128, 512], F32, tag="ps", name=f"bank{gi}")
                    for gi in range(len(groups))
                ]
                # passes 0-4: lhsT (c,ci)->co for tap (kh, kw=c), offset 12kh
                # passes 5-9: K=32 c=0 copy, tap (kh, kw=4), offset 12kh+4
                passes = [
                    (128, 12 * kh, w2a[:, 128 * kh : 128 * kh + 128])
                    for kh in range(5)
                ] + [
                    (32, 12 * kh + 4, w2b[:, 128 * kh : 128 * kh + 128])
                    for kh in range(5)
                ]
                for p, (kk, off, lhsT) in enumerate(passes):
                    for gi, (j, hh) in enumerate(groups):
                        goff = j * H1F + hh * 8 * 144
                        rhs = bass.AP(
                            tensor=h1r[:].tensor,
                            offset=h1r[:].offset + goff + off,
                            ap=[[4 * H1F + 8, kk], [144, 8], [12, 8], [1, 8]],
                        )
                        nc.tensor.matmul(
                            out=banks[gi][:],
                            lhsT=lhsT,
                            rhs=rhs,
                            start=(p == 0),
                            stop=(p == 9),
                        )
                # pool conv2 8x8 -> 4x4 in two stages per group
                for gi, (j, hh) in enumerate(groups):
                    g = 2 * j + hh
                    st1 = S.tile([64, 256], F32, tag="st1")
                    iv = banks[gi][0:64, :].rearrange(
                        "p (soh pw s0) -> p soh pw s0", pw=4, s0=2
                    )
                    nc.vector.tensor_reduce(out=st1[:], in_=iv, axis=AX, op=MAXOP)
                    outv = bass.AP(
                        tensor=h2s[:].tensor,
                        offset=h2s[:].offset + g * 8,
                        ap=[[1024, 64], [256, 4], [64, 4], [1, 8]],
                    )
                    ia = bass.AP(
                        tensor=st1[:].tensor,
                        offset=st1[:].offset,
                        ap=[[256, 64], [8, 4], [1, 4], [32, 8]],
                    )
                    ib = bass.AP(
                        tensor=st1[:].tensor,
                        offset=st1[:].offset + 4,
                        ap=[[256, 64], [8, 4], [1, 4], [32, 8]],
                    )
                    nc.vector.tensor_tensor(out=outv, in0=ia, in1=ib, op=MAXOP)
            nc.scalar.activation(h2s[:], h2s[:], RELU, bias=b2[:])
            dst = bass.AP(
                tensor=h2d[:].tensor,
                offset=h2d[:].offset + blk * 64,
                ap=[[16 * BPC, 64], [BPC, 16], [1, 64]],
            )
            src = bass.AP(
                tensor=h2s[:].tensor,
                offset=h2s[:].offset,
                ap=[[1024, 64], [64, 16], [1, 64]],
            )
            nc.scalar.dma_start(out=dst, in_=src)

        # ---- prologue: prefetch x for blocks 0 and 1, warm up the PE ----
        xreps = {0: load_xrep(0), 1: load_xrep(1)}
        for _ in range(HEAT):
            psh = PS.tile([128, 512], F32, tag="ps")
            nc.tensor.matmul(
                out=psh[:], lhsT=heat[:, 0:128], rhs=heat[:], start=True, stop=True
            )
        h1p0 = conv1(xreps.pop(0))
        reads = {0: build_h1r(h1p0)}

        # ---- pipelined block loop: conv1(k+1) then conv2(k) ----
        for k in range(NBLK):
            if k + 1 < NBLK:
                h1p = conv1(xreps.pop(k + 1))
                reads[k + 1] = build_h1r(h1p)
            if k + 2 < NBLK:
                xreps[k + 2] = load_xrep(k + 2)
            conv2(k, reads.pop(k))

        # ---- fc1: 8 contiguous k-chunk reads from h2d ----
        hr = []
        engs = [nc.sync, nc.gpsimd, nc.scalar]
        for kch in range(8):
            t = P.tile([128, BPC], BF16, tag=f"hr{kch}")
            src = bass.AP(
                tensor=h2d[:].tensor,
                offset=h2d[:].offset + kch * 128 * BPC,
                ap=[[BPC, 128], [1, BPC]],
            )
            engs[kch % 3].dma_start(out=t[:], in_=src)
            hr.append(t)

        h3 = []
        for m in range(4):
            psf = PS.tile([128, 512], F32, tag="ps")
            for kch in range(8):
                nc.tensor.matmul(
                    out=psf[:],
                    lhsT=w3[:, (kch * 4 + m) * 128 : (kch * 4 + m) * 128 + 128],
                    rhs=hr[kch][:],
                    start=(kch == 0),
                    stop=(kch == 7),
                )
            t = P.tile([128, BPC], F32, tag=f"h3{m}")
            nc.scalar.activation(t[:], psf[:], RELU, bias=b3[:, m : m + 1])
            h3.append(t)

        # ---- fc2 + log_softmax, batch on partitions ----
        for bc in range(4):
            psl = PS.tile([128, 10], F32, tag="ps")
            for kch in range(4):
                nc.tensor.matmul(
                    out=psl[:],
                    lhsT=h3[kch][:, bc * 128 : bc * 128 + 128],
                    rhs=fc2[:, kch * 10 : kch * 10 + 10],
                    start=(kch == 0),
                    stop=False,
                )
            nc.tensor.matmul(
                out=psl[:],
                lhsT=ones1[:],
                rhs=fc2b[:],
                start=False,
                stop=True,
            )
            negm = S.tile([128, 1], F32, tag="negm")
            nc.vector.tensor_reduce(
                out=negm[:], in_=psl[:], axis=AX, op=MAXOP, negate=True
            )
            shifted = S.tile([128, 10], F32, tag="shifted")
            nc.vector.tensor_scalar(
                out=shifted[:], in0=psl[:], scalar1=negm[:], scalar2=None, op0=ADDOP
            )
            ex = S.tile([128, 10], F32, tag="ex")
            se = S.tile([128, 1], F32, tag="se")
            nc.scalar.activation(ex[:], shifted[:], EXP, accum_out=se[:])
            lse = S.tile([128, 1], F32, tag="lse")
            nc.scalar.activation(lse[:], se[:], LN)
            osb = S.tile([128, 10], F32, tag="osb")
            nc.vector.tensor_scalar(
                out=osb[:], in0=shifted[:], scalar1=lse[:], scalar2=None, op0=SUBOP
            )
            nc.sync.dma_start(out=ot[bc * 128 : bc * 128 + 128, :], in_=osb[:])

    nc.finalize()
    return nc


def _prep_weights(inputs):
    """Host-side: densify sketch weights and lay them out for the kernel."""
    h1, h2i, h3i = inputs["hash_idx1"], inputs["hash_idx2"], inputs["hash_idx3"]
    s1, s2, s3 = inputs["sgn1"], inputs["sgn2"], inputs["sgn3"]
    w1, w2, w3 = inputs["w1"], inputs["w2"], inputs["w3"]
    b1, b2, b3 = inputs["b1"], inputs["b2"], inputs["b3"]
    fc2w, fc2b = inputs["fc2_w"], inputs["fc2_b"]

    wc1 = (w1[:, h1] * s1[None, :]).astype(np.float32)            # (32, 25)
    wc2 = (w2[:, h2i] * s2[None, :]).astype(np.float32).reshape(64, 32, 5, 5)
    W3 = (w3[:, h3i] * s3[None, :]).astype(np.float32)            # (512, 1024)

    wc1bd = np.zeros((100, 128), np.float32)
    for j in range(4):
        wc1bd[25 * j : 25 * j + 25, 32 * j : 32 * j + 32] = wc1.T

    # conv2 lhsT packs; output channels duplicated to M=128 (HAM heater)
    w2a5 = np.zeros((128, 5, 64), np.float32)
    for c in range(4):
        for kh in range(5):
            w2a5[32 * c : 32 * c + 32, kh, :] = wc2[:, :, kh, c].T
    w2a5 = np.concatenate([w2a5, w2a5], axis=2).reshape(128, 640)
    w2b5 = np.zeros((32, 5, 64), np.float32)
    for kh in range(5):
        w2b5[:, kh, :] = wc2[:, :, kh, 4].T
    w2b5 = np.concatenate([w2b5, w2b5], axis=2).reshape(32, 640)

    # fc1: lhsT chunk (k,m) = W3.T[128k:128k+128, 128m:128m+128]
    w3sb = np.zeros((128, 8, 4, 128), np.float32)
    W3T = np.ascontiguousarray(W3.T)  # (1024, 512)
    for k in range(8):
        for m in range(4):
            w3sb[:, k, m, :] = W3T[128 * k : 128 * k + 128, 128 * m : 128 * m + 128]
    w3sb = w3sb.reshape(128, 4096)

    fc2sb = np.zeros((128, 4, 10), np.float32)
    for k in range(4):
        fc2sb[:, k, :] = fc2w[:, 128 * k : 128 * k + 128].T
    fc2sb = fc2sb.reshape(128, 40)

    b1r = np.tile(np.asarray(b1, np.float32), 4).reshape(128, 1)
    b3sb = np.asarray(b3, np.float32).reshape(4, 128).T.copy()

    bf = lambda a: np.asarray(a, dtype=ml_dtypes.bfloat16)
    f = lambda a: np.ascontiguousarray(a, dtype=np.float32)
    return {
        "wc1bd": bf(wc1bd),
        "w2a5": bf(w2a5),
        "w2b5": bf(w2b5),
        "w3sb": bf(w3sb),
        "fc2sb": f(fc2sb),
        "b1r": f(b1r),
        "b2": f(np.asarray(b2).reshape(64, 1)),
        "b3sb": f(b3sb),
        "fc2b": f(np.asarray(fc2b).reshape(1, 10)),
    }


def kernel(**inputs):
    out, _ = _run(inputs, trace=False)
    return out


def _run(inputs, trace=False):
    if "nc" not in _CACHE:
        _CACHE["nc"] = _build()
    nc = _CACHE["nc"]

    wmap = _prep_weights(inputs)
    x = np.asarray(inputs["x"], np.float32).reshape(4096, 784)

    in_maps = []
    for c in range(NCORES):
        xs = x[c * BPC : (c + 1) * BPC].reshape(-1)
        xs = np.concatenate([xs, np.zeros(XPAD, np.float32)])
        m = dict(wmap)
        m["x"] = np.asarray(xs, dtype=ml_dtypes.bfloat16)
        in_maps.append(m)

    res = run_bass_kernel_spmd(
        nc, in_maps, core_ids=list(range(NCORES)), trace=trace
    )
    out = np.concatenate([res.results[c]["out"] for c in range(NCORES)], axis=0)
    return out.astype(np.float32), res
